# revision 24
# baseline (speedup 1.0000x reference)
"""Trainium2 Bass kernel for nn_DiffeqSolver: RK4 ODE solver with MLP dynamics.

f(y) = tanh(y@W1 + b1)@W2 + b2;  199 RK4 steps over 4096 trajectories, D=128.

Sharding: data-parallel over B=1024 across 8 cores (B_shard=128, N=512
trajectories/core). Per core the 512 trajectories split into 2 independent
streams of 256 so the serial PE->ACT->PE->DVE chain of one stream overlaps
the other. State kept transposed in SBUF as [D=128 partitions, N free] so
matmul contractions (over D, then H) sit on the partition axis.

Measured evolution (HW exec, 8 cores):
  baseline (fp32 mm1 + f32r+residual mm2)      17.01 ms
  all-f32r, b2 folded into tanh biases,
    batched output DMA, per-stream PSUM banks   2.94 ms
  all-bf16 matmul inputs (this version)

Design notes:
- The RK4 state accumulates in full fp32 in SBUF; matmul INPUTS are rounded
  copies (bf16). Numpy simulation of bf16-rounding every matmul input end to
  end gives 5.4e-3 rel err vs the 2e-2 gate (3.7x margin). fp32 state +
  rounded-input-copies is robust to both RN and truncation rounding modes.
- bf16 (vs f32r) halves LDWEIGHTS and moving-data SBUF traffic. The HW trace
  showed walrus-split LDWEIGHTS instructions contending with matmul operand
  fetch for SBUF bandwidth (213ns clean vs ~580ns contended per matmul),
  making the PE array the 90%-busy bottleneck.
- b2 enters via the tanh bias operand (bias_e = b1 + c_e*W1^T b2, per-step
  host tables) for the eval inputs, and via one DVE add (ybf = y + h*b2) for
  the final combine.
- PSUM accumulation groups reset their ENTIRE bank on start=True, so every
  group owns its bank: hps 1 bank/stream (chunk1 accumulates after chunk0's
  bank reset), z1/z2/z3 1 bank/stream; the output-transpose target reuses
  the stream's z1 slot after ynew's final read. 8 banks exactly.
- Output path: PE transpose (bf16) -> PSUM -> one DVE bf16 copy per stream
  into an SBUF staging tile batching 4 steps -> one strided casting DMA
  (gpsimd-initiated, bf16->fp32) per 4 steps.
"""

import numpy as np
import ml_dtypes

import concourse.bass as bass
import concourse.mybir as mybir
from concourse import tile
from concourse.bass_utils import run_bass_kernel_spmd

S, B, D, H, T = 4, 1024, 128, 256, 200
N_CORES = 8
B_SHARD = B // N_CORES          # 128
N = S * B_SHARD                 # 512 trajectories per core
NS = 256                        # stream width (2 streams per core)
N_STREAMS = N // NS
KBATCH = 16                     # steps per output DMA
F32 = mybir.dt.float32
F32R = mybir.dt.float32r
BF16 = mybir.dt.bfloat16
BF = ml_dtypes.bfloat16

_prog_cache = {}


def _r(ap):
    return ap.bitcast(F32R)


def _build(h_steps):
    nsteps = len(h_steps)
    nc = bass.Bass("TRN2", target_bir_lowering=False, debug=False,
                   num_devices=N_CORES)

    x0 = nc.dram_tensor("x0", [S, B_SHARD, D], F32, kind="ExternalInput").ap()
    w1_d = nc.dram_tensor("w1", [D, H], BF16, kind="ExternalInput").ap()
    # prescaled W2 chunk tensors: [chunk, 128, D]
    w16_d = nc.dram_tensor("w2s16", [2, 128, D], BF16,
                           kind="ExternalInput").ap()
    w13_d = nc.dram_tensor("w2s13", [2, 128, D], BF16,
                           kind="ExternalInput").ap()
    b1col_d = nc.dram_tensor("b1col", [128, 2], F32, kind="ExternalInput").ap()
    # per-step tanh biases: b1 + (h/2) W1^T b2 and b1 + h W1^T b2, as
    # [128(part), chunk, step]
    bh_d = nc.dram_tensor("biash", [128, 2, nsteps], F32,
                          kind="ExternalInput").ap()
    bf_d = nc.dram_tensor("biasf", [128, 2, nsteps], F32,
                          kind="ExternalInput").ap()
    # final-combine b2 term: h*b2 per step, [D, nsteps]
    b2f_d = nc.dram_tensor("b2full", [D, nsteps], F32,
                           kind="ExternalInput").ap()
    identf_d = nc.dram_tensor("identf", [128, 128], F32,
                              kind="ExternalInput").ap()
    yout = nc.dram_tensor("yout_t", [nsteps + 1, D, N], BF16,
                          kind="ExternalOutput").ap()

    AF = mybir.ActivationFunctionType
    OP = mybir.AluOpType

    with tile.TileContext(nc) as tc:
        with (
            tc.tile_pool(name="const", bufs=1) as cpool,
            tc.tile_pool(name="state", bufs=3) as spool,
            tc.tile_pool(name="work", bufs=6) as wpool,
            tc.tile_pool(name="acts", bufs=6) as apool,
            tc.tile_pool(name="outcp", bufs=2) as opool,
            tc.tile_pool(name="ph0", bufs=1, space="PSUM") as ph0_pool,
            tc.tile_pool(name="ph1", bufs=1, space="PSUM") as ph1_pool,
            tc.tile_pool(name="pz1", bufs=1, space="PSUM") as pz1_pool,
            tc.tile_pool(name="pz2", bufs=1, space="PSUM") as pz2_pool,
            tc.tile_pool(name="pz3", bufs=1, space="PSUM") as pz3_pool,
        ):
            ph_pools = [ph0_pool, ph1_pool]
            z_pools = {"z1": pz1_pool, "z2": pz2_pool, "z3": pz3_pool}
            # ---- constants ----
            w1_sb = cpool.tile([D, H], BF16, tag="w1")
            nc.sync.dma_start(out=w1_sb[:], in_=w1_d)
            w16 = cpool.tile([128, 2, D], BF16, tag="w16")
            nc.sync.dma_start(out=w16[:],
                              in_=w16_d.rearrange("c k d -> k c d"))
            w13 = cpool.tile([128, 2, D], BF16, tag="w13")
            nc.sync.dma_start(out=w13[:],
                              in_=w13_d.rearrange("c k d -> k c d"))
            b1col = cpool.tile([128, 2], F32, tag="b1col")
            nc.sync.dma_start(out=b1col[:], in_=b1col_d)
            biash = cpool.tile([128, 2, nsteps], F32, tag="biash")
            nc.sync.dma_start(out=biash[:], in_=bh_d)
            biasf = cpool.tile([128, 2, nsteps], F32, tag="biasf")
            nc.sync.dma_start(out=biasf[:], in_=bf_d)
            b2full = cpool.tile([D, nsteps], F32, tag="b2full")
            nc.sync.dma_start(out=b2full[:], in_=b2f_d)
            identf = cpool.tile([128, 128], F32, tag="identf")
            nc.sync.dma_start(out=identf[:], in_=identf_d)

            # ---- initial state: load [b,d] tiles, t=0 output, transpose ----
            x0v = x0.rearrange("s b d -> (s b) d")  # n = s*128 + b
            cur = []
            for st in range(N_STREAMS):
                y0 = spool.tile([D, NS], F32, tag=f"Y{st}")
                tp0 = pz2_pool.tile([128, 2, 128], F32, tag=f"z2_{st}",
                                    name=f"tp_init_{st}")
                for c in range(NS // 128):
                    n0 = st * NS + c * 128
                    xin = wpool.tile([128, D], F32, tag="xin")
                    nc.sync.dma_start(out=xin[:], in_=x0v[n0:n0 + 128, :])
                    nc.tensor.transpose(tp0[:, c, :], xin[:], identf[:])
                nc.vector.tensor_copy(
                    out=y0.rearrange("p (c x) -> p c x", c=2),
                    in_=tp0[:])
                yr0 = wpool.tile([D, NS], BF16, tag=f"Yr{st}")
                nc.vector.tensor_copy(
                    out=yr0.rearrange("p (c x) -> p c x", c=2),
                    in_=tp0[:])
                nc.sync.dma_start(
                    out=yout[0, :, st * NS:(st + 1) * NS], in_=yr0[:])
                cur.append((y0, yr0))

            # ---- time loop (fully unrolled, stream B emitted 2 eval-phases
            #      behind stream A) ----
            wmain = (w16, w13, w13, w16)

            def bias_ap(e, c, i):
                if e == 0:
                    return b1col[:, c:c + 1]
                if e == 3:
                    return biasf[:, c, i:i + 1]
                return biash[:, c, i:i + 1]

            def eval_phase(S_, e):
                """One RK4 eval for stream-state S_: h matmuls (bf16), tanh
                with per-eval bias, z accumulation, DVE combines."""
                st, i = S_["st"], S_["i"]
                if e < 3:
                    zname = ("z1", "z2", "z3")[e]
                    S_[zname] = z_pools[zname].tile(
                        [128, NS], F32, tag=f"{zname}_{st}",
                        name=f"{zname}_{st}_{i}")
                    bank = S_[zname]
                else:
                    bank = S_["z1"]
                rhs = S_["Yr"] if e == 0 else S_["yt"]
                hps = ph_pools[st].tile([128, 2 * NS], F32, tag=f"h{st}")
                a = apool.tile([128, 2 * NS], BF16, tag=f"a{st}")
                for c in range(2):
                    reg = hps[:, c * NS:(c + 1) * NS]
                    nc.tensor.matmul(
                        reg, w1_sb[:, c * 128:(c + 1) * 128],
                        rhs[:], start=(c == 0), stop=(c == 1),
                        skip_group_check=True)
                    nc.scalar.activation(
                        a[:, c * NS:(c + 1) * NS],
                        reg, AF.Tanh, bias=bias_ap(e, c, i))
                for c in range(2):
                    a_ap = a[:, c * NS:(c + 1) * NS]
                    first = (e != 3) and c == 0
                    # z1's accumulation group stays open from e0 (k1) through
                    # e3 (k4); z2/z3 are 2-matmul groups.
                    last = (e != 0) and c == 1
                    nc.tensor.matmul(
                        bank[:], wmain[e][:, c, :], a_ap,
                        start=first, stop=last, skip_group_check=True)
                q, s3, s15 = S_["q"], S_["s3"], S_["s15"]
                if e == 0:
                    ybf = wpool.tile([D, NS], F32, tag=f"ybf{st}")
                    nc.vector.tensor_scalar(ybf[:], S_["Y"][:],
                                            b2full[:, i:i + 1], None,
                                            op0=OP.add)
                    S_["ybf"] = ybf
                if e < 3:
                    yt = wpool.tile([D, NS], BF16, tag=f"yt{st}")
                    sc = (s3, s15, s3)[e]
                    nc.vector.scalar_tensor_tensor(
                        yt[:], bank[:], sc, S_["Y"][:],
                        op0=OP.mult, op1=OP.add)
                    S_["yt"] = yt
                if e == 1:
                    c1 = wpool.tile([D, NS], F32, tag=f"c{st}")
                    nc.vector.scalar_tensor_tensor(
                        c1[:], S_["z2"][:], q, S_["ybf"][:],
                        op0=OP.mult, op1=OP.add)
                    S_["c1"] = c1
                elif e == 2:
                    c2 = wpool.tile([D, NS], F32, tag=f"c{st}")
                    nc.vector.scalar_tensor_tensor(
                        c2[:], S_["z3"][:], q, S_["c1"][:],
                        op0=OP.mult, op1=OP.add)
                    S_["c2"] = c2
                elif e == 3:
                    # bf16 state copy FIRST: it feeds next step's matmul1
                    # (critical path); the fp32 state update can lag.
                    yr = wpool.tile([D, NS], BF16, tag=f"Yr{st}")
                    nc.vector.scalar_tensor_tensor(
                        yr[:], S_["z1"][:], q, S_["c2"][:],
                        op0=OP.mult, op1=OP.add)
                    S_["yrnew"] = yr
                    ynew = spool.tile([D, NS], F32, tag=f"Y{st}")
                    nc.vector.scalar_tensor_tensor(
                        ynew[:], S_["z1"][:], q, S_["c2"][:],
                        op0=OP.mult, op1=OP.add)
                    S_["ynew"] = ynew
                    # output: DMA the bf16 state copy straight out in
                    # [t, d, n] layout; the host undoes the transpose.
                    nc.sync.dma_start(
                        out=yout[i + 1, :, st * NS:(st + 1) * NS],
                        in_=yr[:])

            def new_state(st, i, Y, Yr, stg):
                hf = np.float32(h_steps[i])
                return {
                    "st": st, "i": i, "Y": Y, "Yr": Yr, "stg": stg,
                    "s3": float(np.float32(3.0) * hf / np.float32(0.05)),
                    "s15": float(np.float32(1.5) * hf / np.float32(0.05)),
                    "q": float(hf / np.float32(0.05)),
                }

            # Uniform lag-2 interleave: slots A0 B2' A1 B3' A2 B0 A3 B1
            # keep each engine's in-order queue fed with alternating-stream
            # work spaced by the true dependency distance (2 slots).
            SA = new_state(0, 0, *cur[0], None)
            SB_prev = None
            for i in range(nsteps):
                if i > 0:
                    SA = new_state(0, i, SA["ynew"], SA["yrnew"], None)
                eval_phase(SA, 0)
                if SB_prev is not None:
                    eval_phase(SB_prev, 2)
                eval_phase(SA, 1)
                if SB_prev is not None:
                    eval_phase(SB_prev, 3)   # finish B's previous step
                    SB = new_state(1, i, SB_prev["ynew"],
                                   SB_prev["yrnew"], None)
                else:
                    SB = new_state(1, 0, *cur[1], None)
                eval_phase(SA, 2)
                eval_phase(SB, 0)
                eval_phase(SA, 3)
                eval_phase(SB, 1)
                SB_prev = SB
            eval_phase(SB_prev, 2)
            eval_phase(SB_prev, 3)

    _split_multiwait_instructions(nc)
    return nc


def _split_multiwait_instructions(nc, max_waits=1):
    """This walrus build rejects >1 sync-wait on CTRL-class instructions
    (Tile's exit Drain carries one wait per live semaphore). N waits on one
    instruction == N single-wait NOPs then the instruction, for same-engine
    in-order execution. Mutate nc.m in place before compile."""
    counter = [0]
    for fn in nc.m.functions:
        for bb in fn.blocks:
            new_instructions = []
            for ins in bb.instructions:
                si = getattr(ins, "sync_info", None)
                if si is not None and si.on_wait and len(si.on_wait) > max_waits:
                    for w in si.on_wait[max_waits:]:
                        counter[0] += 1
                        new_instructions.append(mybir.InstNoOp(
                            name=f"I-drainfix-{counter[0]}",
                            engine=ins.engine, ins=[], outs=[],
                            sync_info=mybir.SyncInfo(on_wait=[w], on_update=[]),
                        ))
                    si.on_wait = si.on_wait[:max_waits]
                new_instructions.append(ins)
            bb.instructions = new_instructions


def kernel(first_point, time_steps_to_predict, W1, b1, W2, b2):
    first_point = np.ascontiguousarray(first_point, dtype=np.float32)
    ts = np.asarray(time_steps_to_predict, dtype=np.float32)
    W1 = np.asarray(W1, dtype=np.float32)
    b1 = np.asarray(b1, dtype=np.float32)
    W2 = np.asarray(W2, dtype=np.float32)
    b2 = np.asarray(b2, dtype=np.float32)

    h_steps = (ts[1:] - ts[:-1]).astype(np.float32)
    key = h_steps.tobytes()
    if key not in _prog_cache:
        _prog_cache[key] = _build(list(h_steps))
    nc = _prog_cache[key]

    c16 = np.float32(0.05) / np.float32(6.0)
    c13 = np.float32(0.05) / np.float32(3.0)
    w2s16 = np.stack([c16 * W2[0:128, :], c16 * W2[128:256, :]]
                     ).astype(np.float32)
    w2s13 = np.stack([c13 * W2[0:128, :], c13 * W2[128:256, :]]
                     ).astype(np.float32)
    b1col = np.stack([b1[0:128], b1[128:256]], axis=1).astype(np.float32)
    w1b2 = (W1.T @ b2).astype(np.float32)          # [H]
    w1b2col = np.stack([w1b2[0:128], w1b2[128:256]], axis=1)  # [128, 2]
    halves = (h_steps * np.float32(0.5)).astype(np.float32)
    biash = (b1col[:, :, None] + w1b2col[:, :, None] * halves[None, None, :]
             ).astype(np.float32)
    biasf = (b1col[:, :, None] + w1b2col[:, :, None] * h_steps[None, None, :]
             ).astype(np.float32)
    b2full = (b2[:, None] * h_steps[None, :]).astype(np.float32)

    shared = {
        "w1": W1.astype(BF), "w2s16": w2s16.astype(BF),
        "w2s13": w2s13.astype(BF), "b1col": b1col,
        "biash": biash, "biasf": biasf, "b2full": b2full,
        "identf": np.eye(128, dtype=np.float32),
        "identb": np.eye(128).astype(BF),
    }

    in_maps = []
    for i in range(N_CORES):
        m = dict(shared)
        m["x0"] = np.ascontiguousarray(
            first_point[:, i * B_SHARD:(i + 1) * B_SHARD, :])
        in_maps.append(m)

    import os
    trace = os.environ.get("BASS_KERNEL_PROFILE", "") == "1"
    res = run_bass_kernel_spmd(nc, in_maps, list(range(N_CORES)), trace=trace)
    global last_exec_time_ns, last_result
    last_exec_time_ns = res.exec_time_ns
    last_result = res

    out = np.empty((S, B, len(ts), D), dtype=np.float32)
    for i in range(N_CORES):
        # yout_t is [T, D, N] bf16 with n = s*128 + b; undo the transpose
        yt_ = np.asarray(res.results[i]["yout_t"])
        out[:, i * B_SHARD:(i + 1) * B_SHARD] = (
            yt_.reshape(len(ts), D, S, B_SHARD)
            .transpose(2, 3, 0, 1).astype(np.float32))
    return out
